# revision 1
# baseline (speedup 1.0000x reference)
"""GATReduce Trainium2 kernel (8-core SPMD, data-parallel over nodes).

Reference computation (per node n, head h, feature f):
    a[n,d,h] = a1[n,h] + a2[n,d,h]
    e = softmax_d(leaky_relu(a, 0.01))
    out[n,h,f] = sum_d e[n,d,h] * ft[n,d,h,f]

Shapes: N=16384 nodes, D=32 mailbox, H=8 heads, F=64 features. fp32.

(d, nl) partition layout, nl = n % 4 within a 128-node tile:

  * Every SBUF tensor in the hot path lives with partition p = d*4 + nl.
    For fixed (n, d) the full (h, f) slab of ft is 2048 B contiguous in
    HBM, so every ft DMA descriptor is a 2048 B line (4x the (d,h2)
    layout's 512 B), and out DMAs write 2048 B lines too.
  * The softmax over d is a PARTITION-dim reduction here, done with tiny
    one-hot matmuls instead of PE transposes:
      - a1 is replicated over d by matmul(repmat4[4,128], a1T[4,256])
      - p = exp(lrelu(a1 + a2)) via DVE add + DVE lrelu + ScalarE Exp
      - sum_d via matmul(onehot4[128,4], p) -> s[4, 256]
      - r = 1/s on the DVE, broadcast back by matmul(repmat4, r)
      - e = p * r_rep on the DVE; e is ALREADY in ft's layout, no
        transpose or eT copy needed.
    All softmax math is fp32; only q = e (x) ft is written as bf16 by the
    DVE (free dtype cast on the multiply), making the one-hot reduction
    matmul 1 cyc/row instead of fp32's 4.
  * Reduction over d: onehot4[p, m] = 1[p%4 == m] contracts all 32 d per
    nl in one bf16 matmul of 512 moving cols; col-tiling (tile_position=
    (0,32k)) stacks 4 node-quads per PSUM bank; ScalarE drains; 4 out
    DMAs per tile (one per clump) with 2048 B lines.
  * a2 must stream in (d, nl) layout as 32 B lines (h-runs); it rides the
    GPSIMD SWDGE ring so descriptor generation never blocks ft's ring.

Engine budget per core (cost model): DMA ~420 us (128 MB ft + 2 MB a2 @
~330 GB/s), DVE ~295 us (the e (x) ft multiply is 1 elem/cyc/lane fp32),
PE ~115 us, ScalarE ~105 us.

Measured on 8 axon trn2 cores: 291 us/iter (vs 510-561 us for the
(d,h2)-layout baseline; the 512 B ft lines there held DMA to ~240 GB/s,
2048 B lines sustain ~460 GB/s).  Variants that measured WORSE, all via
schedule/engine effects (the tile schedule around this structure is a
sharp local optimum): GPSIMD offload of part of the multiply (719 us --
Pool tensor_tensor runs ~6-8x below its cost-model rate), ScalarE
fp32->bf16 ft cast + DVE 2x_1p multiply via materialized e-pairs (841
us), ft DMAs split across sync+scalar HWDGE queues (525 us), and the
q-multiply rewritten as scalar_tensor_tensor (ft bypass 0) mult e --
InstTensorScalarPtr lists the fp32 2x_2p DVE perf mode and the cost
model prices it at 0.5 cyc/elem, but a paired interleaved A/B lost to
tensor_tensor in 5/6 rounds (median 675 vs 562 us under co-tenant
load): the mode does not deliver on HW for this op shape.
"""

import numpy as np

import concourse.bacc as bacc
import concourse.bass as bass
import concourse.tile as tile
from concourse import mybir
from concourse.bass_utils import run_bass_kernel_spmd

N_CORES = 8
N, D, H, F = 16384, 32, 8, 64
N_PER_CORE = N // N_CORES  # 2048
TILE_N = 128  # nodes per tile
GROUP_N = 64  # nodes per ft/q working set
NL = 4  # nodes sharing a partition-column group (p = d*NL + nl)
NH_G = GROUP_N // NL  # 16 node-quads per group
NEG_SLOPE = 0.01

_FP = mybir.dt.float32
MM_DT = mybir.dt.bfloat16
# Of the NH_G=16 node-quads in each ft group, this many get their
# e (x) ft multiply on the GPSIMD (Pool) engine instead of the DVE.
# MEASURED: Pool tensor_tensor is ~6-8x slower than its cost-model rate
# (719 us wall at POOL_NH=5 vs 291 us at 0) -- keep at 0.
POOL_NH = 0


def build(
    n_per_core: int = N_PER_CORE,
    reps: int = 1,
    loop_iters: int | None = None,
    internal_ft: bool = False,
    dve_nh: int = None,
    ft_nl: int = None,
) -> bass.Bass:
    # dve_nh / ft_nl: TIMING PROBES ONLY -- shrink the DVE multiply /
    # the ft DMA traffic while keeping everything else identical.  Breaks
    # correctness; used to locate the bottleneck engine.
    assert n_per_core % TILE_N == 0
    n_tiles = n_per_core // TILE_N

    nc = bacc.Bacc(
        "TRN2", target_bir_lowering=False, debug=False, num_devices=N_CORES
    )
    a1_h = nc.declare_dram_parameter("a1", [n_per_core, H, 1], _FP, isOutput=False)
    a2_h = nc.declare_dram_parameter(
        "a2", [n_per_core, D, H, 1], _FP, isOutput=False
    )
    if internal_ft:
        ft_h = nc.dram_tensor("ft_int", [n_per_core, D, H, F], _FP)
    else:
        ft_h = nc.declare_dram_parameter(
            "ft", [n_per_core, D, H, F], _FP, isOutput=False
        )
    onehot_h = nc.declare_dram_parameter("onehot", [128, NL], MM_DT, isOutput=False)
    onehot_f_h = nc.declare_dram_parameter(
        "onehot_f", [128, NL], _FP, isOutput=False
    )
    repmat_h = nc.declare_dram_parameter("repmat", [NL, 128], _FP, isOutput=False)
    out_h = nc.declare_dram_parameter(
        "out", [n_per_core, H, F], _FP, isOutput=True
    )

    with tile.TileContext(nc) as tc:
        import contextlib

        with contextlib.ExitStack() as ctx:
            consts = ctx.enter_context(tc.tile_pool(name="consts", bufs=1))
            a2p = ctx.enter_context(tc.tile_pool(name="a2p", bufs=2))
            smx = ctx.enter_context(tc.tile_pool(name="smx", bufs=3))
            smps = ctx.enter_context(tc.tile_pool(name="smps", bufs=1, space="PSUM"))
            ep = ctx.enter_context(tc.tile_pool(name="ep", bufs=2))
            ftp = ctx.enter_context(tc.tile_pool(name="ftp", bufs=3))
            qp = ctx.enter_context(tc.tile_pool(name="qp", bufs=2))
            pso = ctx.enter_context(tc.tile_pool(name="pso", bufs=4, space="PSUM"))
            outp = ctx.enter_context(tc.tile_pool(name="outp", bufs=2))

            onehot_t = consts.tile([128, NL], MM_DT)
            nc.sync.dma_start(out=onehot_t[:], in_=onehot_h[:])
            onehot_f_t = consts.tile([128, NL], _FP)
            nc.sync.dma_start(out=onehot_f_t[:], in_=onehot_f_h[:])
            repmat_t = consts.tile([NL, 128], _FP)
            nc.sync.dma_start(out=repmat_t[:], in_=repmat_h[:])
            # a1 in (nl)-partition layout: [nl, t, nh, h]
            a1_all = consts.tile([NL, n_tiles, TILE_N // NL, H], _FP)
            nc.sync.dma_start(
                out=a1_all[:],
                in_=a1_h[:].rearrange(
                    "(t nh nl) h one -> nl t nh (h one)", t=n_tiles, nl=NL
                ),
            )

            if loop_iters is not None:
                rep_iter = [None]
                loop_cm = tc.For_i(0, loop_iters, 1)
            else:
                rep_iter = list(range(reps))
                loop_cm = contextlib.nullcontext()

            NHT = TILE_N // NL  # 32 node-quads per tile

            def emit_softmax(t):
                """softmax over d directly in (d, nl)-partition layout.

                Returns e_t [128, NHT, H] fp32 with p = d*4 + nl."""
                n0 = t * TILE_N
                a2_t = a2p.tile([128, NHT, H], _FP)
                # one DMA per nl: DMA APs balance to <= 3 dims, and the
                # (d, nl)-partition gather from [n, d, h] HBM needs 4.
                for nl in range(NL):
                    nc.gpsimd.dma_start(
                        out=a2_t[:].rearrange("(d nl) nh h -> nl d nh h", nl=NL)[
                            nl
                        ],
                        in_=a2_h[n0 : n0 + TILE_N].rearrange(
                            "(nh nl) d h one -> nl d nh (h one)", nl=NL
                        )[nl],
                    )
                # a1 replicated over d: PSUM [128, NHT*H]
                a1rep = smps.tile([128, NHT * H], _FP, tag="a1rep")
                nc.tensor.matmul(
                    a1rep[:],
                    repmat_t[:],
                    a1_all[:, t].rearrange("nl nh h -> nl (nh h)"),
                    start=True,
                    stop=True,
                )
                p_t = smx.tile([128, NHT, H], _FP, tag="p_t")
                nc.vector.tensor_tensor(
                    out=p_t[:],
                    in0=a2_t[:],
                    in1=a1rep[:].rearrange("p (nh h) -> p nh h", h=H),
                    op=mybir.AluOpType.add,
                )
                nc.vector.scalar_tensor_tensor(
                    out=p_t[:],
                    in0=p_t[:],
                    scalar=NEG_SLOPE,
                    in1=p_t[:],
                    op0=mybir.AluOpType.mult,
                    op1=mybir.AluOpType.max,
                )
                nc.scalar.activation(
                    out=p_t[:], in_=p_t[:], func=mybir.ActivationFunctionType.Exp
                )
                # sum over d (partition dim): s[nl, (nh h)]
                s_ps = smps.tile([NL, NHT * H], _FP, tag="s_ps")
                nc.tensor.matmul(
                    s_ps[:],
                    onehot_f_t[:],
                    p_t[:].rearrange("p nh h -> p (nh h)"),
                    start=True,
                    stop=True,
                )
                r_t = smx.tile([NL, NHT * H], _FP, tag="r_t")
                nc.vector.reciprocal(out=r_t[:], in_=s_ps[:])
                # broadcast r back over d: PSUM [128, NHT*H]
                r_rep = smps.tile([128, NHT * H], _FP, tag="r_rep")
                nc.tensor.matmul(
                    r_rep[:], repmat_t[:], r_t[:], start=True, stop=True
                )
                e_t = ep.tile([128, NHT, H], _FP, tag="e_t")
                nc.vector.tensor_tensor(
                    out=e_t[:],
                    in0=p_t[:],
                    in1=r_rep[:].rearrange("p (nh h) -> p nh h", h=H),
                    op=mybir.AluOpType.mult,
                )
                return e_t

            def emit_tile(t, e_t):
                n0 = t * TILE_N
                out_t = outp.tile([128, TILE_N // 16, 512], _FP)
                for g in range(TILE_N // GROUP_N):  # groups of 64 nodes
                    gn0 = n0 + g * GROUP_N
                    ft_t = ftp.tile([128, NH_G, H * F], _FP)
                    # one DMA per nl (3-dim APs); 2048 B lines
                    for nl in range(NL if ft_nl is None else ft_nl):
                        nc.sync.dma_start(
                            out=ft_t[:].rearrange(
                                "(d nl) nh hf -> nl d nh hf", nl=NL
                            )[nl],
                            in_=ft_h[gn0 : gn0 + GROUP_N].rearrange(
                                "(nh nl) d h f -> nl d nh (h f)", nl=NL
                            )[nl],
                        )
                    q_t = qp.tile([128, NH_G, H, F], MM_DT)
                    ft_v = ft_t[:].rearrange("p nh (h f) -> p nh h f", f=F)
                    e_g = e_t[:, g * NH_G : (g + 1) * NH_G, :]
                    nd = NH_G - POOL_NH if dve_nh is None else dve_nh
                    nc.vector.tensor_tensor(
                        out=q_t[:, :nd],
                        in0=ft_v[:, :nd],
                        in1=e_g[:, :nd]
                        .unsqueeze(-1)
                        .to_broadcast((128, nd, H, F)),
                        op=mybir.AluOpType.mult,
                    )
                    if POOL_NH:
                        nc.gpsimd.tensor_tensor(
                            out=q_t[:, nd:],
                            in0=ft_v[:, nd:],
                            in1=e_g[:, nd:]
                            .unsqueeze(-1)
                            .to_broadcast((128, POOL_NH, H, F)),
                            op=mybir.AluOpType.mult,
                        )
                    q_v = q_t[:].rearrange("p nh h f -> p nh (h f)")
                    for quarter in range(NH_G // 4):  # PSUM tile = 16 nodes
                        r = g * (NH_G // 4) + quarter
                        ps = pso.tile([128, 512], _FP)
                        for k in range(4):
                            nh = quarter * 4 + k
                            if dve_nh is not None:
                                nh = nh % dve_nh  # probe: reread written rows
                            nc.tensor.matmul(
                                ps[32 * k : 32 * k + NL, :],
                                onehot_t[:],
                                q_v[:, nh],
                                start=True,
                                stop=True,
                                tile_position=(0, 32 * k),
                            )
                        nc.scalar.copy(out=out_t[:, r, :], in_=ps[:])

                # out DMA: clump k rows 32k+nl hold node 16*r+4k+nl at free
                # offset r*512 (r = g*4+quarter); (h f) = 2048 B lines.
                for k in range(4):
                    dst = out_h[n0 : n0 + TILE_N].rearrange(
                        "(r kk nl) h f -> kk nl r (h f)", kk=4, nl=NL
                    )[k]
                    nc.scalar.dma_start(
                        out=dst,
                        in_=out_t[32 * k : 32 * k + NL],
                    )

            with loop_cm:
                for _ in rep_iter:
                    for t in range(n_tiles):
                        e_t = emit_softmax(t)
                        emit_tile(t, e_t)

    nc.compile()
    return nc


def _make_consts():
    import ml_dtypes

    onehot = np.zeros((128, NL), dtype=np.float32)
    onehot[np.arange(128), np.arange(128) % NL] = 1.0
    repmat = onehot.T.copy()
    return onehot.astype(ml_dtypes.bfloat16), onehot, repmat


def run(
    a1: np.ndarray,
    a2: np.ndarray,
    ft: np.ndarray,
    n_per_core: int = N_PER_CORE,
    reps: int = 1,
    nc: bass.Bass | None = None,
):
    if nc is None:
        nc = build(n_per_core, reps)
    onehot, onehot_f, repmat = _make_consts()
    ft_names = {
        a.memorylocations[0].name
        for a in nc.m.functions[0].allocations
        if getattr(a, "kind", None) == "ExternalInput"
    }
    in_maps = []
    for c in range(N_CORES):
        sl = slice(c * n_per_core, (c + 1) * n_per_core)
        m = {
            "a1": np.ascontiguousarray(a1[sl]),
            "a2": np.ascontiguousarray(a2[sl]),
            "onehot": onehot,
            "onehot_f": onehot_f,
            "repmat": repmat,
        }
        if "ft" in ft_names:
            m["ft"] = np.ascontiguousarray(ft[sl])
        in_maps.append(m)
    res = run_bass_kernel_spmd(nc, in_maps, list(range(N_CORES)))
    out = np.concatenate([res.results[c]["out"] for c in range(N_CORES)], axis=0)
    return out


def kernel(a1: np.ndarray, a2: np.ndarray, ft: np.ndarray) -> np.ndarray:
    a1 = np.asarray(a1, dtype=np.float32)
    a2 = np.asarray(a2, dtype=np.float32)
    ft = np.asarray(ft, dtype=np.float32)
    assert a1.shape == (N, H, 1) and a2.shape == (N, D, H, 1)
    assert ft.shape == (N, D, H, F)
    out = run(a1.reshape(N, H), a2.reshape(N, D, H), ft)
    return out.astype(np.float32)

